# revision 1
# baseline (speedup 1.0000x reference)
"""Trainium2 Bass kernel for the HMM forward-algorithm problem.

Strategy
--------
The reference does, per time step, a log-domain matrix-vector product
  alpha_t[b,k] = em[b,t,k] + logsumexp_j(alpha_{t-1}[b,j] + tran[j,k])
followed by logsumexp_k.  We run the whole recurrence in *probability*
domain on the TensorEngine:

  phat_t = E_t  *  (phat_{t-1} @ P)          (elementwise * matmul)

where P = softmax(tran) rows (constant) and E_t = exp(em_t - kappa) with a
global shift kappa that keeps E <= ~1.  phat decays by ~e^-3 per step, so we
renormalise every RN steps by the previous column sum (dumping the exact
bf16 scale factor used so the host can undo it).  The per-step
logsumexp_k(alpha_t) output reduces to log(sum_k phat_t) + known offsets;
sum_k phat is computed on the TensorEngine with a ones-vector matmul and
streamed to an output strip.  The final log / cumsum / length-indexing is
tiny (T x B) and done on the host in float64.

Emissions: em[b,t,h] = 0.25 * sum_s x[s,h,obs[b,t,s]] - L[h], where
x is the raw emission table and L[h] = 0.25*sum_s logsumexp_v x[s,h,:].
The host pre-transposes x to a (S*V, H) bf16 row table; the device gathers
rows with indirect DMA (128 rows = 16 timesteps x 8 batch), sums the 4
sources, transposes 128x128 blocks on the TensorEngine to H-major and
applies exp(0.25*x - L - kappa) on the ScalarEngine directly into the
E-strip consumed by the scan.

Sharding: data-parallel over batch (8 of 64 rows per core).  Tables are
replicated.  No collectives.
"""
import sys

sys.path.insert(0, "/opt/trn_rl_repo")

import numpy as np
import ml_dtypes

import concourse.bass as bass
import concourse.bacc as bacc
import concourse.tile as tile
import concourse.mybir as mybir
import concourse.bass_utils as bass_utils
from concourse.masks import make_identity

B, T, S, H, V = 64, 512, 4, 512, 10000
NC = 8            # cores
BL = B // NC      # batch rows per core
P_ = 128          # partitions
HCN = H // P_     # h chunks
TBLK = 16         # timesteps per gather block
RN = 8            # renorm interval
F32 = mybir.dt.float32
BF16 = mybir.dt.bfloat16
I32 = mybir.dt.int32
EXP = mybir.ActivationFunctionType.Exp
MULT = mybir.AluOpType.mult

_compiled = {}


def _n_renorms(t_steps):
    return len([t for t in range(1, t_steps) if t % RN == 0])


def build(t_steps=T):
    """Build + bacc-compile the per-core Bass program (identical on all cores)."""
    nblk = t_steps // TBLK
    nc = bacc.Bacc("TRN2", target_bir_lowering=False, debug=False,
                   enable_asserts=False, num_devices=NC)

    tabt = nc.dram_tensor("tabt", [S * V, H], BF16, kind="ExternalInput").ap()
    pm_d = nc.dram_tensor("pm", [P_, HCN * HCN * P_], BF16, kind="ExternalInput").ap()
    idx_d = nc.dram_tensor("idx", [P_, S * nblk], I32, kind="ExternalInput").ap()
    bias_d = nc.dram_tensor("bias", [P_, HCN], F32, kind="ExternalInput").ap()
    expp_d = nc.dram_tensor("expp", [P_, HCN], F32, kind="ExternalInput").ap()
    rstrip_d = nc.dram_tensor("rstrip", [1, t_steps * BL], F32,
                              kind="ExternalOutput").ap()
    nrn = max(1, _n_renorms(t_steps))
    rinv_d = nc.dram_tensor("rinvstrip", [1, nrn * BL], F32,
                            kind="ExternalOutput").ap()

    with tile.TileContext(nc) as tc:
        with (tc.tile_pool(name="const", bufs=1) as cp,
              tc.tile_pool(name="estrip", bufs=nblk) as ep,
              tc.tile_pool(name="gath", bufs=6) as gp,
              tc.tile_pool(name="xsum", bufs=2) as xp,
              tc.tile_pool(name="phat", bufs=3) as pp,
              tc.tile_pool(name="small", bufs=4) as sp,
              tc.tile_pool(name="qpsum", bufs=2, space="PSUM") as qp,
              tc.tile_pool(name="rpsum", bufs=2, space="PSUM") as rp,
              tc.tile_pool(name="tpsum", bufs=2, space="PSUM") as tp_,
              tc.tile_pool(name="ipsum", bufs=2, space="PSUM") as ip):

            # ---- constants ----
            pm_t = cp.tile([P_, HCN * HCN * P_], BF16, name="pmt")
            nc.sync.dma_start(pm_t[:, :], pm_d[:, :])
            idx_t = cp.tile([P_, S * nblk], I32, name="idxt")
            nc.sync.dma_start(idx_t[:, :], idx_d[:, :])
            bias_t = cp.tile([P_, HCN], F32, name="biast")
            nc.sync.dma_start(bias_t[:, :], bias_d[:, :])
            expp_t = cp.tile([P_, HCN], F32, name="exppt")
            nc.sync.dma_start(expp_t[:, :], expp_d[:, :])
            ones128 = cp.tile([P_, 1], BF16, name="ones128")
            nc.gpsimd.memset(ones128[:, :], 1.0)
            onesrow = cp.tile([1, P_], BF16, name="onesrow")
            nc.gpsimd.memset(onesrow[:, :], 1.0)
            ident = cp.tile([P_, P_], F32, name="ident")
            make_identity(nc, ident[:, :])
            rstrip_t = cp.tile([1, t_steps * BL], F32, name="rstript")
            rinv_t = cp.tile([1, nrn * BL], F32, name="rinvt")

            eb_list = [None] * nblk

            def gather_block(blk):
                gs = []
                for s in range(S):
                    g = gp.tile([P_, H], BF16, tag="g", name=f"g{blk}_{s}")
                    col = s * nblk + blk
                    nc.gpsimd.indirect_dma_start(
                        out=g[:, :], out_offset=None, in_=tabt[:, :],
                        in_offset=bass.IndirectOffsetOnAxis(
                            ap=idx_t[:, col:col + 1], axis=0))
                    gs.append(g)
                x01 = xp.tile([P_, H], F32, tag="x01", name=f"x01_{blk}")
                nc.vector.tensor_add(x01[:, :], gs[0][:, :], gs[1][:, :])
                x23 = xp.tile([P_, H], F32, tag="x23", name=f"x23_{blk}")
                nc.vector.tensor_add(x23[:, :], gs[2][:, :], gs[3][:, :])
                x = xp.tile([P_, H], F32, tag="x", name=f"x_{blk}")
                nc.vector.tensor_add(x[:, :], x01[:, :], x23[:, :])
                eb = ep.tile([P_, TBLK * HCN * BL], BF16, tag="eb",
                             name=f"eb{blk}")
                eb4 = eb.rearrange("p (t c b) -> p t c b", t=TBLK, c=HCN)
                for c in range(HCN):
                    tpp = tp_.tile([P_, P_], F32, tag="tp")
                    nc.tensor.transpose(out=tpp[:, :],
                                        in_=x[:, c * P_:(c + 1) * P_],
                                        identity=ident[:, :])
                    nc.scalar.activation(
                        eb4[:, :, c, :],
                        tpp.rearrange("p (t b) -> p t b", t=TBLK),
                        EXP, bias=bias_t[:, c:c + 1], scale=0.25)
                return eb

            def rgroup(pprev, r_slot):
                r1 = rp.tile([1, BL], F32, tag="r1")
                for jc in range(HCN):
                    nc.tensor.matmul(r1[:, :], lhsT=ones128[:, :],
                                     rhs=pprev[:, jc * BL:(jc + 1) * BL],
                                     start=(jc == 0), stop=(jc == HCN - 1))
                nc.scalar.copy(rstrip_t[:, r_slot * BL:(r_slot + 1) * BL],
                               r1[:, :])
                return r1

            # ---- first gather block + phat_0 init ----
            eb_list[0] = gather_block(0)
            eb0_4 = eb_list[0].rearrange("p (t c b) -> p t c b", t=TBLK, c=HCN)
            for c in range(HCN):
                nc.vector.tensor_scalar_mul(eb0_4[:, 0, c, :],
                                            eb0_4[:, 0, c, :],
                                            expp_t[:, c:c + 1])
            phat = eb_list[0][:, 0:HCN * BL]

            # ---- interleaved gather + scan ----
            ridx = 0
            for blk in range(nblk):
                if blk + 1 < nblk:
                    eb_list[blk + 1] = gather_block(blk + 1)
                t_lo = max(1, blk * TBLK)
                for t in range(t_lo, (blk + 1) * TBLK):
                    renorm = (t % RN == 0)
                    r1 = rgroup(phat, t - 1)
                    q = qp.tile([P_, HCN * BL], F32, tag="q")
                    for kc in range(HCN):
                        for jc in range(HCN):
                            nc.tensor.matmul(
                                q[:, kc * BL:(kc + 1) * BL],
                                lhsT=pm_t[:, (jc * HCN + kc) * P_:
                                          (jc * HCN + kc + 1) * P_],
                                rhs=phat[:, jc * BL:(jc + 1) * BL],
                                start=(jc == 0), stop=(jc == HCN - 1))
                    if renorm:
                        rinv32 = sp.tile([1, BL], F32, tag="rinv32")
                        nc.vector.reciprocal(rinv32[:, :], r1[:, :])
                        rinvbf = sp.tile([1, BL], BF16, tag="rinvbf")
                        nc.vector.tensor_copy(rinvbf[:, :], rinv32[:, :])
                        nc.scalar.copy(rinv_t[:, ridx * BL:(ridx + 1) * BL],
                                       rinvbf[:, :])
                        rinv_ps = ip.tile([P_, BL], F32, tag="rinvps")
                        nc.tensor.matmul(rinv_ps[:, :], lhsT=onesrow[:, :],
                                         rhs=rinvbf[:, :],
                                         start=True, stop=True)
                        ridx += 1
                    ebt = eb_list[t // TBLK]
                    base = (t % TBLK) * HCN * BL
                    pnew = pp.tile([P_, HCN * BL], BF16, tag="ph")
                    nc.vector.tensor_tensor(
                        pnew[:, :], q[:, :],
                        ebt[:, base: base + HCN * BL], MULT)
                    if renorm:
                        for kc in range(HCN):
                            cs = slice(kc * BL, (kc + 1) * BL)
                            nc.vector.tensor_tensor(pnew[:, cs], pnew[:, cs],
                                                    rinv_ps[:, :], MULT)
                    phat = pnew

            rgroup(phat, t_steps - 1)
            nc.sync.dma_start(rstrip_d[:, :], rstrip_t[:, :])
            nc.sync.dma_start(rinv_d[:, :], rinv_t[:, :])

    nc.compile()
    return nc


def _get_compiled(t_steps=T):
    if t_steps not in _compiled:
        _compiled[t_steps] = build(t_steps)
    return _compiled[t_steps]


def _host_prep(obs, emis, tran, priors, t_steps):
    """Returns (shared_inputs, per_core_idx, kappa)."""
    nblk = t_steps // TBLK
    # transition softmax -> bf16 chunk layout [j, (jc*HCN+kc)*128 + k]
    m = tran.max(axis=1, keepdims=True)
    e = np.exp(tran - m, dtype=np.float32)
    P = (e / e.sum(axis=1, keepdims=True)).astype(ml_dtypes.bfloat16)
    pm = np.ascontiguousarray(
        P.reshape(HCN, P_, HCN, P_).transpose(1, 0, 2, 3).reshape(P_, -1))

    # transposed bf16 emission table, rows indexed by s*V+v
    tabT = np.ascontiguousarray(
        emis.transpose(0, 2, 1)).astype(ml_dtypes.bfloat16).reshape(S * V, H)

    # L[h] and kappa
    mx = emis.max(axis=2)                                   # (S,H)
    lse = mx + np.log(np.exp(emis - mx[:, :, None],
                             dtype=np.float32).sum(axis=2))
    L = 0.25 * lse.sum(axis=0)                              # (H,)
    kap_h = 0.25 * mx.sum(axis=0) - L
    kappa = float(kap_h.max())
    bias = np.ascontiguousarray(
        (-(L + kappa)).astype(np.float32).reshape(HCN, P_).T)   # (128,4)
    expp = np.ascontiguousarray(
        np.exp(priors, dtype=np.float32).reshape(HCN, P_).T)    # (128,4)

    # per-core gather row indices: idx[p=(tt*BL+bb), s*nblk+blk]
    per_core_idx = []
    svec = (np.arange(S, dtype=np.int64) * V)
    for c in range(NC):
        o = obs[c * BL:(c + 1) * BL, :t_steps, :]           # (BL,t,S)
        o = o + svec[None, None, :]
        o = o.transpose(1, 0, 2)                            # (t, BL, S)
        o = o.reshape(nblk, TBLK, BL, S)
        o = o.transpose(1, 2, 3, 0).reshape(TBLK * BL, S * nblk)
        per_core_idx.append(np.ascontiguousarray(o.astype(np.int32)))

    shared = {"tabt": tabT, "pm": pm, "bias": bias, "expp": expp}
    return shared, per_core_idx, kappa


def _host_post(results, lengths, kappa, t_steps):
    nrn = max(1, _n_renorms(t_steps))
    ans = np.zeros((B, 1), np.float32)
    tt = np.arange(t_steps, dtype=np.float64)
    for c in range(NC):
        r = results[c]["rstrip"].reshape(t_steps, BL).astype(np.float64)
        rinv = results[c]["rinvstrip"].reshape(nrn, BL).astype(np.float64)
        rho_log = np.zeros((t_steps, BL), np.float64)
        k = 0
        for t in range(1, t_steps):
            if t % RN == 0:
                rho_log[t] = np.log(rinv[k])
                k += 1
        logsums = np.log(r) + (tt[:, None] + 1.0) * kappa \
            - np.cumsum(rho_log, axis=0)
        lens = np.clip(lengths[c * BL:(c + 1) * BL], 1, t_steps)
        ans[c * BL:(c + 1) * BL, 0] = logsums[
            lens - 1, np.arange(BL)].astype(np.float32)
    return ans


def run(inputs, t_steps=T, trace=False):
    obs = np.asarray(inputs["obs"])
    lengths = np.asarray(inputs["lengths"])
    emis = np.asarray(inputs["unnormalized_emis"], np.float32)
    tran = np.asarray(inputs["unnormalized_tran"], np.float32)
    priors = np.asarray(inputs["log_state_priors"], np.float32)

    nc = _get_compiled(t_steps)
    shared, per_core_idx, kappa = _host_prep(obs, emis, tran, priors, t_steps)
    in_maps = [dict(shared, idx=per_core_idx[c]) for c in range(NC)]
    res = bass_utils.run_bass_kernel_spmd(nc, in_maps,
                                          core_ids=list(range(NC)),
                                          trace=trace)
    ans = _host_post(res.results, lengths, kappa, t_steps)
    return ans, res


def kernel(obs, lengths, unnormalized_emis, unnormalized_tran,
           log_state_priors):
    ans, _ = run(dict(obs=obs, lengths=lengths,
                      unnormalized_emis=unnormalized_emis,
                      unnormalized_tran=unnormalized_tran,
                      log_state_priors=log_state_priors))
    return ans



# revision 3
# speedup vs baseline: 1.5538x; 1.5538x over previous
"""Trainium2 Bass kernel for the HMM forward-algorithm problem.

Strategy
--------
The reference does, per time step, a log-domain matrix-vector product
  alpha_t[b,k] = em[b,t,k] + logsumexp_j(alpha_{t-1}[b,j] + tran[j,k])
followed by logsumexp_k.  We run the whole recurrence in *probability*
domain on the TensorEngine:

  phat_t = E_t  *  (phat_{t-1} @ P)          (elementwise * matmul)

where P = softmax(tran) rows (constant) and E_t = exp(em_t - kappa) with a
global shift kappa that keeps E <= ~1.  phat decays by ~e^-3 per step, so we
renormalise every RN steps by an earlier column sum (dumping the exact f32
scale used so the host can undo it).  The recurrence is a 512-link serial
chain PE -> (PSUM latency) -> DVE multiply -> (latency) -> PE, so everything
else is kept OFF that chain:

- renorm: the reciprocal/broadcast/E-scale are prepared 4 steps ahead and
  folded into the E-strip slice, so renorm steps cost nothing on the chain;
- per-step column sums (for the per-t logsumexp output) accumulate into a
  PSUM strip of RN slots, copied out by the Act engine once per RN steps;
- emission gathers: indirect DMA fetches bf16 rows; the 4 sources are
  summed via matmul-by-identity transposes accumulating in PSUM (run in PE
  idle windows), then Act applies exp(0.25*x - L - kappa) into the E-strip.

Emissions: em[b,t,h] = 0.25 * sum_s x[s,h,obs[b,t,s]] - L[h], where
x is the raw emission table and L[h] = 0.25*sum_s logsumexp_v x[s,h,:].
The host pre-transposes x to a (S*V, H) bf16 row table; the device gathers
rows with indirect DMA (128 rows = 16 timesteps x 8 batch per source).

Sharding: data-parallel over batch (8 of 64 rows per core).  Tables are
replicated.  No collectives.  Final log / cumsum / length-indexing is tiny
(T x B) and done on the host in float64.
"""
import sys

sys.path.insert(0, "/opt/trn_rl_repo")

import numpy as np
import ml_dtypes

import concourse.bass as bass
import concourse.bacc as bacc
import concourse.tile as tile
import concourse.mybir as mybir
import concourse.bass_utils as bass_utils
from concourse.masks import make_identity

B, T, S, H, V = 64, 512, 4, 512, 10000
NC = 8            # cores
BL = B // NC      # batch rows per core
P_ = 128          # partitions
HCN = H // P_     # h chunks
TBLK = 16         # timesteps per gather block
RN = 8            # renorm interval
F32 = mybir.dt.float32
BF16 = mybir.dt.bfloat16
I32 = mybir.dt.int32
EXP = mybir.ActivationFunctionType.Exp
MULT = mybir.AluOpType.mult

_compiled = {}


def _n_renorms(t_steps):
    return len([t for t in range(1, t_steps) if t % RN == 0])


def build(t_steps=T):
    """Build + bacc-compile the per-core Bass program (identical on all cores)."""
    nblk = t_steps // TBLK
    nc = bacc.Bacc("TRN2", target_bir_lowering=False, debug=False,
                   enable_asserts=False, num_devices=NC)

    tabt = nc.dram_tensor("tabt", [S * V, H], BF16, kind="ExternalInput").ap()
    pm_d = nc.dram_tensor("pm", [P_, HCN * HCN * P_], BF16, kind="ExternalInput").ap()
    idx_d = nc.dram_tensor("idx", [P_, S * nblk], I32, kind="ExternalInput").ap()
    bias_d = nc.dram_tensor("bias", [P_, HCN], F32, kind="ExternalInput").ap()
    bias2_d = nc.dram_tensor("bias2", [P_, HCN], F32, kind="ExternalInput").ap()
    rstrip_d = nc.dram_tensor("rstrip", [1, t_steps * BL], F32,
                              kind="ExternalOutput").ap()
    nrn = max(1, _n_renorms(t_steps))
    rinv_d = nc.dram_tensor("rinvstrip", [1, nrn * BL], F32,
                            kind="ExternalOutput").ap()

    with tile.TileContext(nc) as tc:
        with (tc.tile_pool(name="const", bufs=1) as cp,
              tc.tile_pool(name="estrip", bufs=nblk) as ep,
              tc.tile_pool(name="gath", bufs=5) as gp,
              tc.tile_pool(name="phat", bufs=4) as pp,
              tc.tile_pool(name="small", bufs=2) as sp,
              tc.tile_pool(name="ebr", bufs=2) as er,
              tc.tile_pool(name="qpsum", bufs=2, space="PSUM") as qp,
              tc.tile_pool(name="rstripps", bufs=2, space="PSUM") as rp,
              tc.tile_pool(name="r2psum", bufs=1, space="PSUM") as r2p,
              tc.tile_pool(name="rbpsum", bufs=1, space="PSUM") as rbp,
              tc.tile_pool(name="tpsum", bufs=2, space="PSUM") as tp_):

            # ---- constants ----
            pm_t = cp.tile([P_, HCN * HCN * P_], BF16, name="pmt")
            nc.sync.dma_start(pm_t[:, :], pm_d[:, :])
            idx_t = cp.tile([P_, S * nblk], I32, name="idxt")
            nc.sync.dma_start(idx_t[:, :], idx_d[:, :])
            bias_t = cp.tile([P_, HCN], F32, name="biast")
            nc.sync.dma_start(bias_t[:, :], bias_d[:, :])
            bias2_t = cp.tile([P_, HCN], F32, name="bias2t")
            nc.sync.dma_start(bias2_t[:, :], bias2_d[:, :])
            ones128 = cp.tile([P_, 1], BF16, name="ones128")
            nc.gpsimd.memset(ones128[:, :], 1.0)
            onesrow_f = cp.tile([1, P_], F32, name="onesrowf")
            nc.gpsimd.memset(onesrow_f[:, :], 1.0)
            identb = cp.tile([P_, P_], BF16, name="identb")
            make_identity(nc, identb[:, :])
            rstrip_t = cp.tile([1, t_steps * BL], F32, name="rstript")
            rinv_t = cp.tile([1, nrn * BL], F32, name="rinvt")

            eb_list = [None] * nblk
            g_list = [None] * nblk

            def emit_gather(blk):
                gs = []
                for s in range(S):
                    g = gp.tile([P_, H], BF16, tag="g", name=f"g{blk}_{s}")
                    col = s * nblk + blk
                    nc.gpsimd.indirect_dma_start(
                        out=g[:, :], out_offset=None, in_=tabt[:, :],
                        in_offset=bass.IndirectOffsetOnAxis(
                            ap=idx_t[:, col:col + 1], axis=0))
                    gs.append(g)
                g_list[blk] = gs
                eb_list[blk] = ep.tile([P_, TBLK * HCN * BL], BF16, tag="eb",
                                       name=f"eb{blk}")

            def emit_chunk(blk, c):
                # transpose the 4 source gathers for h-chunk c, summing in
                # PSUM, then exp into the E-strip on the Act engine
                gs = g_list[blk]
                tpp = tp_.tile([P_, P_], F32, tag="tp")
                for s in range(S):
                    nc.tensor.matmul(tpp[:, :],
                                     lhsT=gs[s][:, c * P_:(c + 1) * P_],
                                     rhs=identb[:, :],
                                     start=(s == 0), stop=(s == S - 1))
                eb4 = eb_list[blk].rearrange("p (t c b) -> p t c b",
                                             t=TBLK, c=HCN)
                nc.scalar.activation(
                    eb4[:, :, c, :],
                    tpp.rearrange("p (t b) -> p t b", t=TBLK),
                    EXP, bias=bias_t[:, c:c + 1], scale=0.25)
                return tpp

            # ---- block 0: gather, transpose, E-strip, and phat_0 ----
            emit_gather(0)
            phat = pp.tile([P_, HCN * BL], BF16, tag="ph", name="phat0")
            for c in range(HCN):
                tpp = emit_chunk(0, c)
                nc.scalar.activation(phat[:, c * BL:(c + 1) * BL],
                                     tpp[:, 0:BL],
                                     EXP, bias=bias2_t[:, c:c + 1], scale=0.25)

            # ---- interleaved gather + scan ----
            ridx = 0
            rps = None
            pend_r2 = None
            pend_tiled = None
            pend_rb = None
            ebr_cur = None
            last_rn = (t_steps - 1) // RN * RN  # last renorm step < t_steps

            def rgroup(ph, u):
                # column sums of phat_u into PSUM r-strip slot u%RN
                nonlocal rps
                if u % RN == 0:
                    rps = rp.tile([1, RN * BL], F32, tag="rstrip")
                slot = u % RN
                for jc in range(HCN):
                    nc.tensor.matmul(rps[:, slot * BL:(slot + 1) * BL],
                                     lhsT=ones128[:, :],
                                     rhs=ph[:, jc * BL:(jc + 1) * BL],
                                     start=(jc == 0), stop=(jc == HCN - 1))
                if slot == RN - 1:
                    g = u // RN
                    nc.scalar.copy(
                        rstrip_t[:, g * RN * BL:(g + 1) * RN * BL], rps[:, :])

            for t in range(1, t_steps):
                blk = t // TBLK
                j = t % TBLK
                m = t % RN
                tr = t - m + RN          # next renorm step after t
                prep = (m >= 4 and tr <= last_rn)

                # PE: q = P^T phat (16 matmuls, 4 accumulation groups)
                q = qp.tile([P_, HCN * BL], F32, tag="q")
                for kc in range(HCN):
                    for jc in range(HCN):
                        nc.tensor.matmul(
                            q[:, kc * BL:(kc + 1) * BL],
                            lhsT=pm_t[:, (jc * HCN + kc) * P_:
                                      (jc * HCN + kc + 1) * P_],
                            rhs=phat[:, jc * BL:(jc + 1) * BL],
                            start=(jc == 0), stop=(jc == HCN - 1))
                # PE (off-chain): column sums of phat_{t-1} into slot t-1
                rgroup(phat, t - 1)
                # PE (off-chain): renorm scale source = column sums of phat
                if prep and m == 4:
                    pend_r2 = r2p.tile([1, BL], F32, tag="r2")
                    for jc in range(HCN):
                        nc.tensor.matmul(pend_r2[:, :], lhsT=ones128[:, :],
                                         rhs=phat[:, jc * BL:(jc + 1) * BL],
                                         start=(jc == 0), stop=(jc == HCN - 1))
                # PE (off-chain): broadcast rinv over partitions
                if prep and m == 6:
                    pend_rb = rbp.tile([P_, HCN * BL], F32, tag="rb")
                    nc.tensor.matmul(pend_rb[:, :], lhsT=onesrow_f[:, :],
                                     rhs=pend_tiled[:, :],
                                     start=True, stop=True)
                # Pool: prefetch next block's gathers
                if j == 1 and blk + 1 < nblk:
                    emit_gather(blk + 1)
                # PE/Act (off-chain): transpose+exp bursts for next block
                if blk + 1 < nblk and 10 <= j <= 13:
                    emit_chunk(blk + 1, j - 10)

                # DVE: the chain multiply
                pnew = pp.tile([P_, HCN * BL], BF16, tag="ph")
                if m == 0 and ebr_cur is not None:
                    nc.vector.tensor_tensor(pnew[:, :], q[:, :],
                                            ebr_cur[:, :], MULT)
                    ebr_cur = None
                else:
                    base = j * HCN * BL
                    nc.vector.tensor_tensor(
                        pnew[:, :], q[:, :],
                        eb_list[blk][:, base: base + HCN * BL], MULT)

                # DVE/Act (off-chain): renorm preparation pipeline
                if prep and m == 5:
                    pend_tiled = sp.tile([1, HCN * BL], F32, tag="tiled")
                    nc.vector.reciprocal(pend_tiled[:, 0:BL], pend_r2[:, :])
                    nc.scalar.copy(pend_tiled[:, BL:2 * BL],
                                   pend_tiled[:, 0:BL])
                    nc.scalar.copy(pend_tiled[:, 2 * BL:4 * BL],
                                   pend_tiled[:, 0:2 * BL])
                    nc.scalar.copy(rinv_t[:, ridx * BL:(ridx + 1) * BL],
                                   pend_tiled[:, 0:BL])
                    ridx += 1
                if prep and m == 7:
                    nb = tr // TBLK
                    nbase = (tr % TBLK) * HCN * BL
                    ebr_cur = er.tile([P_, HCN * BL], BF16, tag="ebr")
                    nc.vector.tensor_tensor(
                        ebr_cur[:, :],
                        eb_list[nb][:, nbase: nbase + HCN * BL],
                        pend_rb[:, :], MULT)

                phat = pnew

            rgroup(phat, t_steps - 1)
            nc.sync.dma_start(rstrip_d[:, :], rstrip_t[:, :])
            nc.sync.dma_start(rinv_d[:, :], rinv_t[:, :])

    nc.compile()
    return nc


def _get_compiled(t_steps=T):
    if t_steps not in _compiled:
        _compiled[t_steps] = build(t_steps)
    return _compiled[t_steps]


def _host_prep(obs, emis, tran, priors, t_steps):
    """Returns (shared_inputs, per_core_idx, kappa)."""
    nblk = t_steps // TBLK
    # transition softmax -> bf16 chunk layout [j, (jc*HCN+kc)*128 + k]
    m = tran.max(axis=1, keepdims=True)
    e = np.exp(tran - m, dtype=np.float32)
    P = (e / e.sum(axis=1, keepdims=True)).astype(ml_dtypes.bfloat16)
    pm = np.ascontiguousarray(
        P.reshape(HCN, P_, HCN, P_).transpose(1, 0, 2, 3).reshape(P_, -1))

    # transposed bf16 emission table, rows indexed by s*V+v
    tabT = np.ascontiguousarray(
        emis.transpose(0, 2, 1)).astype(ml_dtypes.bfloat16).reshape(S * V, H)

    # L[h] and kappa
    mx = emis.max(axis=2)                                   # (S,H)
    lse = mx + np.log(np.exp(emis - mx[:, :, None],
                             dtype=np.float32).sum(axis=2))
    L = 0.25 * lse.sum(axis=0)                              # (H,)
    kap_h = 0.25 * mx.sum(axis=0) - L
    kappa = float(kap_h.max())
    bias = np.ascontiguousarray(
        (-(L + kappa)).astype(np.float32).reshape(HCN, P_).T)   # (128,4)
    bias2 = np.ascontiguousarray(
        (-(L + kappa) + priors).astype(np.float32).reshape(HCN, P_).T)

    # per-core gather row indices: idx[p=(tt*BL+bb), s*nblk+blk]
    per_core_idx = []
    svec = (np.arange(S, dtype=np.int64) * V)
    for c in range(NC):
        o = obs[c * BL:(c + 1) * BL, :t_steps, :]           # (BL,t,S)
        o = o + svec[None, None, :]
        o = o.transpose(1, 0, 2)                            # (t, BL, S)
        o = o.reshape(nblk, TBLK, BL, S)
        o = o.transpose(1, 2, 3, 0).reshape(TBLK * BL, S * nblk)
        per_core_idx.append(np.ascontiguousarray(o.astype(np.int32)))

    shared = {"tabt": tabT, "pm": pm, "bias": bias, "bias2": bias2}
    return shared, per_core_idx, kappa


def _host_post(results, lengths, kappa, t_steps):
    nrn = max(1, _n_renorms(t_steps))
    ans = np.zeros((B, 1), np.float32)
    tt = np.arange(t_steps, dtype=np.float64)
    for c in range(NC):
        r = results[c]["rstrip"].reshape(t_steps, BL).astype(np.float64)
        rinv = results[c]["rinvstrip"].reshape(nrn, BL).astype(np.float64)
        rho_log = np.zeros((t_steps, BL), np.float64)
        k = 0
        for t in range(1, t_steps):
            if t % RN == 0:
                rho_log[t] = np.log(rinv[k])
                k += 1
        logsums = np.log(r) + (tt[:, None] + 1.0) * kappa \
            - np.cumsum(rho_log, axis=0)
        lens = np.clip(lengths[c * BL:(c + 1) * BL], 1, t_steps)
        ans[c * BL:(c + 1) * BL, 0] = logsums[
            lens - 1, np.arange(BL)].astype(np.float32)
    return ans


def run(inputs, t_steps=T, trace=False):
    obs = np.asarray(inputs["obs"])
    lengths = np.asarray(inputs["lengths"])
    emis = np.asarray(inputs["unnormalized_emis"], np.float32)
    tran = np.asarray(inputs["unnormalized_tran"], np.float32)
    priors = np.asarray(inputs["log_state_priors"], np.float32)

    nc = _get_compiled(t_steps)
    shared, per_core_idx, kappa = _host_prep(obs, emis, tran, priors, t_steps)
    in_maps = [dict(shared, idx=per_core_idx[c]) for c in range(NC)]
    res = bass_utils.run_bass_kernel_spmd(nc, in_maps,
                                          core_ids=list(range(NC)),
                                          trace=trace)
    ans = _host_post(res.results, lengths, kappa, t_steps)
    return ans, res


def kernel(obs, lengths, unnormalized_emis, unnormalized_tran,
           log_state_priors):
    ans, _ = run(dict(obs=obs, lengths=lengths,
                      unnormalized_emis=unnormalized_emis,
                      unnormalized_tran=unnormalized_tran,
                      log_state_priors=log_state_priors))
    return ans


# revision 5
# speedup vs baseline: 1.5688x; 1.0096x over previous
"""Trainium2 Bass kernel for the HMM forward-algorithm problem.

Strategy
--------
The reference does, per time step, a log-domain matrix-vector product
  alpha_t[b,k] = em[b,t,k] + logsumexp_j(alpha_{t-1}[b,j] + tran[j,k])
followed by logsumexp_k.  We run the whole recurrence in *probability*
domain on the TensorEngine:

  phat_t = E_t  *  (phat_{t-1} @ P)          (elementwise * matmul)

where P = softmax(tran) rows (constant) and E_t = exp(em_t - kappa) with a
global shift kappa that keeps E <= ~1.  phat decays by ~e^-3 per step, so we
renormalise every RN steps by an earlier column sum (dumping the exact f32
scale used so the host can undo it).

The recurrence is a T-link serial chain PE -> (PSUM latency) -> DVE multiply
-> (latency) -> PE whose per-link latency is fixed-cost dominated, so the 8
batch rows per core are split into TWO independent 4-row chains that
interleave: each chain's link is cheaper and the engines stay busy with the
other chain during latency gaps.  Everything else is kept OFF the chains:

- renorm: the reciprocal/broadcast/E-scale are prepared 5+ steps ahead and
  folded into a pre-scaled E-strip slice, so renorm steps cost nothing;
- per-step column sums (the per-t logsumexp output) accumulate into a PSUM
  strip of RN slots, copied out by the Act engine once per RN steps;
- emission gathers: indirect DMA fetches bf16 rows two blocks ahead; the 4
  sources are summed via matmul-by-identity transposes accumulating in PSUM
  (PE idle windows), then Act applies exp(0.25*x - L - kappa) into the
  E-strip.

Emissions: em[b,t,h] = 0.25 * sum_s x[s,h,obs[b,t,s]] - L[h], where
x is the raw emission table and L[h] = 0.25*sum_s logsumexp_v x[s,h,:].
The host pre-transposes x to a (S*V, H) bf16 row table; the device gathers
rows with indirect DMA (128 rows = 16 timesteps x 8 batch per source).

Sharding: data-parallel over batch (8 of 64 rows per core).  Tables are
replicated.  No collectives.  Final log / cumsum / length-indexing is tiny
(T x B) and done on the host in float64.
"""
import sys

sys.path.insert(0, "/opt/trn_rl_repo")

import numpy as np
import ml_dtypes

import concourse.bass as bass
import concourse.bacc as bacc
import concourse.tile as tile
import concourse.mybir as mybir
import concourse.bass_utils as bass_utils
from concourse.masks import make_identity

B, T, S, H, V = 64, 512, 4, 512, 10000
NC = 8            # cores
BL = B // NC      # batch rows per core
NG = 2            # independent chains per core
BG = BL // NG     # batch rows per chain
P_ = 128          # partitions
HCN = H // P_     # h chunks
TBLK = 16         # timesteps per gather block
RN = 8            # renorm interval
F32 = mybir.dt.float32
BF16 = mybir.dt.bfloat16
I32 = mybir.dt.int32
EXP = mybir.ActivationFunctionType.Exp
MULT = mybir.AluOpType.mult

_compiled = {}


def _n_renorms(t_steps):
    return len([t for t in range(1, t_steps) if t % RN == 0])


def build(t_steps=T):
    """Build + bacc-compile the per-core Bass program (identical on all cores)."""
    nblk = t_steps // TBLK
    nc = bacc.Bacc("TRN2", target_bir_lowering=False, debug=False,
                   enable_asserts=False, num_devices=NC)

    tabt = nc.dram_tensor("tabt", [S * V, H], BF16, kind="ExternalInput").ap()
    pm_d = nc.dram_tensor("pm", [P_, HCN * HCN * P_], BF16, kind="ExternalInput").ap()
    idx_d = nc.dram_tensor("idx", [P_, S * nblk], I32, kind="ExternalInput").ap()
    bias_d = nc.dram_tensor("bias", [P_, HCN], F32, kind="ExternalInput").ap()
    bias2_d = nc.dram_tensor("bias2", [P_, HCN], F32, kind="ExternalInput").ap()
    rstrip_d = nc.dram_tensor("rstrip", [1, t_steps * BL], F32,
                              kind="ExternalOutput").ap()
    nrn = max(1, _n_renorms(t_steps))
    rinv_d = nc.dram_tensor("rinvstrip", [1, nrn * BL], F32,
                            kind="ExternalOutput").ap()

    with tile.TileContext(nc) as tc:
        with (tc.tile_pool(name="const", bufs=1) as cp,
              tc.tile_pool(name="estrip", bufs=nblk) as ep,
              tc.tile_pool(name="gath", bufs=9) as gp,
              tc.tile_pool(name="phat", bufs=3) as pp,
              tc.tile_pool(name="small", bufs=2) as sp,
              tc.tile_pool(name="ebr", bufs=2) as er,
              tc.tile_pool(name="qpsum", bufs=2, space="PSUM") as qp,
              tc.tile_pool(name="rstripps", bufs=2, space="PSUM") as rp,
              tc.tile_pool(name="combops", bufs=1, space="PSUM") as cbp,
              tc.tile_pool(name="tpsum", bufs=1, space="PSUM") as tp_):

            # ---- constants ----
            pm_t = cp.tile([P_, HCN * HCN * P_], BF16, name="pmt")
            nc.sync.dma_start(pm_t[:, :], pm_d[:, :])
            idx_t = cp.tile([P_, S * nblk], I32, name="idxt")
            nc.sync.dma_start(idx_t[:, :], idx_d[:, :])
            bias_t = cp.tile([P_, HCN], F32, name="biast")
            nc.sync.dma_start(bias_t[:, :], bias_d[:, :])
            bias2_t = cp.tile([P_, HCN], F32, name="bias2t")
            nc.sync.dma_start(bias2_t[:, :], bias2_d[:, :])
            ones128 = cp.tile([P_, 1], BF16, name="ones128")
            nc.gpsimd.memset(ones128[:, :], 1.0)
            onesrow_f = cp.tile([1, P_], F32, name="onesrowf")
            nc.gpsimd.memset(onesrow_f[:, :], 1.0)
            identb = cp.tile([P_, P_], BF16, name="identb")
            make_identity(nc, identb[:, :])
            rstrip_t = cp.tile([1, t_steps * BL], F32, name="rstript")
            rinv_t = cp.tile([1, nrn * BL], F32, name="rinvt")

            eb_list = [None] * nblk
            g_list = [None] * nblk

            def emit_gather(blk):
                gs = []
                for s in range(S):
                    g = gp.tile([P_, H], BF16, tag="g", name=f"g{blk}_{s}")
                    col = s * nblk + blk
                    nc.gpsimd.indirect_dma_start(
                        out=g[:, :], out_offset=None, in_=tabt[:, :],
                        in_offset=bass.IndirectOffsetOnAxis(
                            ap=idx_t[:, col:col + 1], axis=0))
                    gs.append(g)
                g_list[blk] = gs
                eb_list[blk] = ep.tile([P_, TBLK * HCN * BL], BF16, tag="eb",
                                       name=f"eb{blk}")

            def emit_chunk(blk, c):
                # transpose the 4 source gathers for h-chunk c, summing in
                # PSUM, then exp into the E-strip on the Act engine
                gs = g_list[blk]
                tpp = tp_.tile([P_, P_], F32, tag="tp")
                for s in range(S):
                    nc.tensor.matmul(tpp[:, :],
                                     lhsT=gs[s][:, c * P_:(c + 1) * P_],
                                     rhs=identb[:, :],
                                     start=(s == 0), stop=(s == S - 1))
                eb4 = eb_list[blk].rearrange("p (t c b) -> p t c b",
                                             t=TBLK, c=HCN)
                nc.scalar.activation(
                    eb4[:, :, c, :],
                    tpp.rearrange("p (t b) -> p t b", t=TBLK),
                    EXP, bias=bias_t[:, c:c + 1], scale=0.25)
                return tpp

            def eb_slice(t, g):
                # [128, (HCN, BG)] E-strip view for chain g at step t
                eb4 = eb_list[t // TBLK].rearrange("p (t c b) -> p t c b",
                                                   t=TBLK, c=HCN)
                return eb4[:, t % TBLK, :, g * BG:(g + 1) * BG]

            # ---- blocks 0/1: gathers; block 0 transposes + phat_0 ----
            emit_gather(0)
            emit_gather(1)
            phat = [pp.tile([P_, HCN * BG], BF16, tag=f"ph{g}",
                            name=f"phat0_{g}") for g in range(NG)]
            for c in range(HCN):
                tpp = emit_chunk(0, c)
                for g in range(NG):
                    nc.scalar.activation(
                        phat[g][:, c * BG:(c + 1) * BG],
                        tpp[:, g * BG:(g + 1) * BG],
                        EXP, bias=bias2_t[:, c:c + 1], scale=0.25)

            # ---- interleaved gather + two-chain scan ----
            # combo PSUM tile columns: rb_g at [g*16:(g+1)*16), r2_g at
            # [32+g*4 : 32+(g+1)*4) on partition 0
            ridx = 0
            rps = None
            combo = None
            tiled = None
            rv8 = None
            ebr_cur = [None, None]
            last_rn = (t_steps - 1) // RN * RN  # last renorm step < t_steps
            CW = HCN * BG                      # rb width per chain (16)

            def rgroup(g, u):
                # column sums of chain g's phat_u into PSUM r-strip slot u%RN
                nonlocal rps
                if u % RN == 0 and g == 0:
                    rps = rp.tile([1, RN * BL], F32, tag="rstrip")
                lo = (u % RN) * BL + g * BG
                for jc in range(HCN):
                    nc.tensor.matmul(rps[:, lo:lo + BG],
                                     lhsT=ones128[:, :],
                                     rhs=phat[g][:, jc * BG:(jc + 1) * BG],
                                     start=(jc == 0), stop=(jc == HCN - 1))

            for t in range(1, t_steps):
                blk = t // TBLK
                j = t % TBLK
                m = t % RN
                tr = t - m + RN          # next renorm step after t
                prep = (m >= 2 and tr <= last_rn)

                # PE: q_g = P^T phat_g (16 matmuls each), then column sums
                qs = []
                for g in range(NG):
                    q = qp.tile([P_, HCN * BG], F32, tag=f"q{g}")
                    for kc in range(HCN):
                        for jc in range(HCN):
                            nc.tensor.matmul(
                                q[:, kc * BG:(kc + 1) * BG],
                                lhsT=pm_t[:, (jc * HCN + kc) * P_:
                                          (jc * HCN + kc + 1) * P_],
                                rhs=phat[g][:, jc * BG:(jc + 1) * BG],
                                start=(jc == 0), stop=(jc == HCN - 1))
                    qs.append(q)
                    rgroup(g, t - 1)
                if (t - 1) % RN == RN - 1:
                    grp = (t - 1) // RN
                    nc.scalar.copy(
                        rstrip_t[:, grp * RN * BL:(grp + 1) * RN * BL],
                        rps[:, :])
                # PE (off-chain): renorm scale source = column sums of phat
                if prep and m == 2:
                    combo = cbp.tile([P_, NG * CW + NG * BG], F32, tag="combo")
                    for g in range(NG):
                        lo = NG * CW + g * BG
                        for jc in range(HCN):
                            nc.tensor.matmul(
                                combo[0:1, lo:lo + BG], lhsT=ones128[:, :],
                                rhs=phat[g][:, jc * BG:(jc + 1) * BG],
                                start=(jc == 0), stop=(jc == HCN - 1))
                # PE (off-chain): broadcast rinv over partitions
                if prep and m == 5:
                    for g in range(NG):
                        nc.tensor.matmul(combo[:, g * CW:(g + 1) * CW],
                                         lhsT=onesrow_f[:, :],
                                         rhs=tiled[:, g * CW:(g + 1) * CW],
                                         start=True, stop=True)
                # Pool: prefetch gathers two blocks ahead
                if j == 14 and blk + 2 < nblk:
                    emit_gather(blk + 2)
                # PE/Act (off-chain): transpose+exp bursts for next block
                if blk + 1 < nblk and 7 <= j <= 10:
                    emit_chunk(blk + 1, j - 7)

                # DVE: the chain multiplies
                for g in range(NG):
                    pnew = pp.tile([P_, HCN * BG], BF16, tag=f"ph{g}")
                    pv = pnew.rearrange("p (c b) -> p c b", c=HCN)
                    qv = qs[g].rearrange("p (c b) -> p c b", c=HCN)
                    if m == 0 and ebr_cur[g] is not None:
                        ev = ebr_cur[g].rearrange("p (c b) -> p c b", c=HCN)
                        ebr_cur[g] = None
                    else:
                        ev = eb_slice(t, g)
                    nc.vector.tensor_tensor(pv[:, :, :], qv[:, :, :],
                                            ev[:, :, :], MULT)
                    phat[g] = pnew

                # DVE/Act (off-chain): renorm preparation pipeline
                if prep and m == 3:
                    rv8 = sp.tile([1, BL], F32, tag="rv8")
                    nc.vector.reciprocal(rv8[:, :],
                                         combo[0:1, NG * CW:NG * CW + BL])
                    nc.scalar.copy(rinv_t[:, ridx * BL:(ridx + 1) * BL],
                                   rv8[:, :])
                    ridx += 1
                    tiled = sp.tile([1, NG * CW], F32, tag="tiled")
                    for g in range(NG):
                        o = g * CW
                        nc.scalar.copy(tiled[:, o:o + BG],
                                       rv8[:, g * BG:(g + 1) * BG])
                        nc.scalar.copy(tiled[:, o + BG:o + 2 * BG],
                                       tiled[:, o:o + BG])
                        nc.scalar.copy(tiled[:, o + 2 * BG:o + 4 * BG],
                                       tiled[:, o:o + 2 * BG])
                if prep and m == 7:
                    for g in range(NG):
                        ebr = er.tile([P_, HCN * BG], BF16, tag=f"ebr{g}")
                        rbv = combo[:, g * CW:(g + 1) * CW].rearrange(
                            "p (c b) -> p c b", c=HCN)
                        nc.vector.tensor_tensor(
                            ebr.rearrange("p (c b) -> p c b", c=HCN)[:, :, :],
                            eb_slice(tr, g)[:, :, :], rbv[:, :, :], MULT)
                        ebr_cur[g] = ebr

            for g in range(NG):
                rgroup(g, t_steps - 1)
            grp = (t_steps - 1) // RN
            nc.scalar.copy(rstrip_t[:, grp * RN * BL:(grp + 1) * RN * BL],
                           rps[:, :])
            nc.sync.dma_start(rstrip_d[:, :], rstrip_t[:, :])
            nc.sync.dma_start(rinv_d[:, :], rinv_t[:, :])

    nc.compile()
    return nc


def _get_compiled(t_steps=T):
    if t_steps not in _compiled:
        _compiled[t_steps] = build(t_steps)
    return _compiled[t_steps]


def _host_prep(obs, emis, tran, priors, t_steps):
    """Returns (shared_inputs, per_core_idx, kappa)."""
    nblk = t_steps // TBLK
    # transition softmax -> bf16 chunk layout [j, (jc*HCN+kc)*128 + k]
    m = tran.max(axis=1, keepdims=True)
    e = np.exp(tran - m, dtype=np.float32)
    P = (e / e.sum(axis=1, keepdims=True)).astype(ml_dtypes.bfloat16)
    pm = np.ascontiguousarray(
        P.reshape(HCN, P_, HCN, P_).transpose(1, 0, 2, 3).reshape(P_, -1))

    # transposed bf16 emission table, rows indexed by s*V+v
    tabT = np.ascontiguousarray(
        emis.transpose(0, 2, 1)).astype(ml_dtypes.bfloat16).reshape(S * V, H)

    # L[h] and kappa
    mx = emis.max(axis=2)                                   # (S,H)
    lse = mx + np.log(np.exp(emis - mx[:, :, None],
                             dtype=np.float32).sum(axis=2))
    L = 0.25 * lse.sum(axis=0)                              # (H,)
    kap_h = 0.25 * mx.sum(axis=0) - L
    kappa = float(kap_h.max())
    bias = np.ascontiguousarray(
        (-(L + kappa)).astype(np.float32).reshape(HCN, P_).T)   # (128,4)
    bias2 = np.ascontiguousarray(
        (-(L + kappa) + priors).astype(np.float32).reshape(HCN, P_).T)

    # per-core gather row indices: idx[p=(tt*BL+bb), s*nblk+blk]
    per_core_idx = []
    svec = (np.arange(S, dtype=np.int64) * V)
    for c in range(NC):
        o = obs[c * BL:(c + 1) * BL, :t_steps, :]           # (BL,t,S)
        o = o + svec[None, None, :]
        o = o.transpose(1, 0, 2)                            # (t, BL, S)
        o = o.reshape(nblk, TBLK, BL, S)
        o = o.transpose(1, 2, 3, 0).reshape(TBLK * BL, S * nblk)
        per_core_idx.append(np.ascontiguousarray(o.astype(np.int32)))

    shared = {"tabt": tabT, "pm": pm, "bias": bias, "bias2": bias2}
    return shared, per_core_idx, kappa


def _host_post(results, lengths, kappa, t_steps):
    nrn = max(1, _n_renorms(t_steps))
    ans = np.zeros((B, 1), np.float32)
    tt = np.arange(t_steps, dtype=np.float64)
    for c in range(NC):
        r = results[c]["rstrip"].reshape(t_steps, BL).astype(np.float64)
        rinv = results[c]["rinvstrip"].reshape(nrn, BL).astype(np.float64)
        rho_log = np.zeros((t_steps, BL), np.float64)
        k = 0
        for t in range(1, t_steps):
            if t % RN == 0:
                rho_log[t] = np.log(rinv[k])
                k += 1
        logsums = np.log(r) + (tt[:, None] + 1.0) * kappa \
            - np.cumsum(rho_log, axis=0)
        lens = np.clip(lengths[c * BL:(c + 1) * BL], 1, t_steps)
        ans[c * BL:(c + 1) * BL, 0] = logsums[
            lens - 1, np.arange(BL)].astype(np.float32)
    return ans


def run(inputs, t_steps=T, trace=False):
    obs = np.asarray(inputs["obs"])
    lengths = np.asarray(inputs["lengths"])
    emis = np.asarray(inputs["unnormalized_emis"], np.float32)
    tran = np.asarray(inputs["unnormalized_tran"], np.float32)
    priors = np.asarray(inputs["log_state_priors"], np.float32)

    nc = _get_compiled(t_steps)
    shared, per_core_idx, kappa = _host_prep(obs, emis, tran, priors, t_steps)
    in_maps = [dict(shared, idx=per_core_idx[c]) for c in range(NC)]
    res = bass_utils.run_bass_kernel_spmd(nc, in_maps,
                                          core_ids=list(range(NC)),
                                          trace=trace)
    ans = _host_post(res.results, lengths, kappa, t_steps)
    return ans, res


def kernel(obs, lengths, unnormalized_emis, unnormalized_tran,
           log_state_priors):
    ans, _ = run(dict(obs=obs, lengths=lengths,
                      unnormalized_emis=unnormalized_emis,
                      unnormalized_tran=unnormalized_tran,
                      log_state_priors=log_state_priors))
    return ans


# revision 8
# speedup vs baseline: 1.5757x; 1.0044x over previous
"""Trainium2 Bass kernel for the HMM forward-algorithm problem.

Strategy
--------
The reference does, per time step, a log-domain matrix-vector product
  alpha_t[b,k] = em[b,t,k] + logsumexp_j(alpha_{t-1}[b,j] + tran[j,k])
followed by logsumexp_k.  We run the whole recurrence in *probability*
domain on the TensorEngine:

  phat_t = E_t  *  (phat_{t-1} @ P)          (elementwise * matmul)

where P = softmax(tran) rows (constant) and E_t = exp(em_t - kappa) with a
global shift kappa that keeps E <= ~1.  phat decays by ~e^-3 per step, so we
renormalise every RN steps by an earlier column sum (dumping the exact f32
scale used so the host can undo it).

The recurrence is a T-link serial chain PE -> (PSUM latency) -> DVE multiply
-> (latency) -> PE whose per-link latency is fixed-cost dominated, so the 8
batch rows per core are split into TWO independent 4-row chains that
interleave: each chain's link is cheaper and the engines stay busy with the
other chain during latency gaps.  Everything else is kept OFF the chains:

- renorm: the reciprocal/broadcast/E-scale are prepared 5+ steps ahead and
  folded into a pre-scaled E-strip slice, so renorm steps cost nothing;
- per-step column sums (the per-t logsumexp output) accumulate into a PSUM
  strip of RN slots, copied out by the Act engine once per RN steps;
- emission gathers: indirect DMA fetches bf16 rows two blocks ahead; the 4
  sources are summed via matmul-by-identity transposes accumulating in PSUM
  (PE idle windows), then Act applies exp(0.25*x - L - kappa) into the
  E-strip.

Emissions: em[b,t,h] = 0.25 * sum_s x[s,h,obs[b,t,s]] - L[h], where
x is the raw emission table and L[h] = 0.25*sum_s logsumexp_v x[s,h,:].
The host pre-transposes x to a (S*V, H) bf16 row table; the device gathers
rows with indirect DMA (128 rows = 16 timesteps x 8 batch per source).

Sharding: data-parallel over batch (8 of 64 rows per core).  Tables are
replicated.  No collectives.  Final log / cumsum / length-indexing is tiny
(T x B) and done on the host in float64.
"""
import sys

sys.path.insert(0, "/opt/trn_rl_repo")

import numpy as np
import ml_dtypes

import concourse.bass as bass
import concourse.bacc as bacc
import concourse.tile as tile
import concourse.mybir as mybir
import concourse.bass_utils as bass_utils
from concourse.masks import make_identity

B, T, S, H, V = 64, 512, 4, 512, 10000
NC = 8            # cores
BL = B // NC      # batch rows per core
NG = 2            # independent chains per core
BG = BL // NG     # batch rows per chain
P_ = 128          # partitions
HCN = H // P_     # h chunks
TBLK = 16         # timesteps per gather block
RN = 8            # renorm interval
F32 = mybir.dt.float32
BF16 = mybir.dt.bfloat16
I32 = mybir.dt.int32
EXP = mybir.ActivationFunctionType.Exp
MULT = mybir.AluOpType.mult

_compiled = {}


def _n_renorms(t_steps):
    return len([t for t in range(1, t_steps) if t % RN == 0])


def build(t_steps=T):
    """Build + bacc-compile the per-core Bass program (identical on all cores)."""
    nblk = t_steps // TBLK
    nc = bacc.Bacc("TRN2", target_bir_lowering=False, debug=False,
                   enable_asserts=False, num_devices=NC)

    tabt = nc.dram_tensor("tabt", [S * V, H], BF16, kind="ExternalInput").ap()
    pm_d = nc.dram_tensor("pm", [P_, HCN * HCN * P_], BF16, kind="ExternalInput").ap()
    idx_d = nc.dram_tensor("idx", [P_, S * nblk], I32, kind="ExternalInput").ap()
    bias_d = nc.dram_tensor("bias", [P_, HCN], F32, kind="ExternalInput").ap()
    bias2_d = nc.dram_tensor("bias2", [P_, HCN], F32, kind="ExternalInput").ap()
    rstrip_d = nc.dram_tensor("rstrip", [1, t_steps * BL], F32,
                              kind="ExternalOutput").ap()
    nrn = max(1, _n_renorms(t_steps))
    rinv_d = nc.dram_tensor("rinvstrip", [1, nrn * BL], F32,
                            kind="ExternalOutput").ap()

    with tile.TileContext(nc) as tc:
        with (tc.tile_pool(name="const", bufs=1) as cp,
              tc.tile_pool(name="estrip", bufs=nblk) as ep,
              tc.tile_pool(name="gath", bufs=9) as gp,
              tc.tile_pool(name="phat", bufs=3) as pp,
              tc.tile_pool(name="small", bufs=2) as sp,
              tc.tile_pool(name="ebr", bufs=2) as er,
              tc.tile_pool(name="qpsum", bufs=2, space="PSUM") as qp,
              tc.tile_pool(name="rstripps", bufs=2, space="PSUM") as rp,
              tc.tile_pool(name="combops", bufs=1, space="PSUM") as cbp,
              tc.tile_pool(name="tpsum", bufs=1, space="PSUM") as tp_):

            # ---- constants ----
            idx_t = cp.tile([P_, S * nblk], I32, name="idxt")
            nc.sync.dma_start(idx_t[:, :], idx_d[:, :])
            pm_t = cp.tile([P_, HCN * HCN * P_], BF16, name="pmt")
            nc.sync.dma_start(pm_t[:, :], pm_d[:, :])
            bias_t = cp.tile([P_, HCN], F32, name="biast")
            nc.sync.dma_start(bias_t[:, :], bias_d[:, :])
            bias2_t = cp.tile([P_, HCN], F32, name="bias2t")
            nc.sync.dma_start(bias2_t[:, :], bias2_d[:, :])
            ones128 = cp.tile([P_, 1], BF16, name="ones128")
            nc.gpsimd.memset(ones128[:, :], 1.0)
            onesrow_f = cp.tile([1, P_], F32, name="onesrowf")
            nc.gpsimd.memset(onesrow_f[:, :], 1.0)
            identb = cp.tile([P_, P_], BF16, name="identb")
            make_identity(nc, identb[:, :])
            rstrip_t = cp.tile([1, t_steps * BL], F32, name="rstript")
            rinv_t = cp.tile([1, nrn * BL], F32, name="rinvt")

            eb_list = [None] * nblk
            g_list = [None] * nblk

            def emit_gather(blk):
                gs = []
                for s in range(S):
                    g = gp.tile([P_, H], BF16, tag="g", name=f"g{blk}_{s}")
                    col = s * nblk + blk
                    nc.gpsimd.indirect_dma_start(
                        out=g[:, :], out_offset=None, in_=tabt[:, :],
                        in_offset=bass.IndirectOffsetOnAxis(
                            ap=idx_t[:, col:col + 1], axis=0))
                    gs.append(g)
                g_list[blk] = gs
                eb_list[blk] = ep.tile([P_, TBLK * HCN * BL], BF16, tag="eb",
                                       name=f"eb{blk}")

            def emit_chunk(blk, c):
                # transpose the 4 source gathers for h-chunk c, summing in
                # PSUM, then exp into the E-strip on the Act engine
                gs = g_list[blk]
                tpp = tp_.tile([P_, P_], F32, tag="tp")
                for s in range(S):
                    nc.tensor.matmul(tpp[:, :],
                                     lhsT=gs[s][:, c * P_:(c + 1) * P_],
                                     rhs=identb[:, :],
                                     start=(s == 0), stop=(s == S - 1))
                eb4 = eb_list[blk].rearrange("p (t c b) -> p t c b",
                                             t=TBLK, c=HCN)
                nc.scalar.activation(
                    eb4[:, :, c, :],
                    tpp.rearrange("p (t b) -> p t b", t=TBLK),
                    EXP, bias=bias_t[:, c:c + 1], scale=0.25)
                return tpp

            def eb_slice(t, g):
                # [128, (HCN, BG)] E-strip view for chain g at step t
                eb4 = eb_list[t // TBLK].rearrange("p (t c b) -> p t c b",
                                                   t=TBLK, c=HCN)
                return eb4[:, t % TBLK, :, g * BG:(g + 1) * BG]

            # ---- blocks 0/1: gathers; block 0 transposes + phat_0 ----
            emit_gather(0)
            emit_gather(1)
            phat = [pp.tile([P_, HCN * BG], BF16, tag=f"ph{g}",
                            name=f"phat0_{g}") for g in range(NG)]
            for c in range(HCN):
                tpp = emit_chunk(0, c)
                for g in range(NG):
                    nc.scalar.activation(
                        phat[g][:, c * BG:(c + 1) * BG],
                        tpp[:, g * BG:(g + 1) * BG],
                        EXP, bias=bias2_t[:, c:c + 1], scale=0.25)

            # ---- interleaved gather + two-chain scan ----
            # combo PSUM tile columns: rb_g at [g*16:(g+1)*16), r2_g at
            # [32+g*4 : 32+(g+1)*4) on partition 0
            ridx = 0
            rps = None
            combo = None
            tiled = None
            rv8 = None
            ebr_cur = [None, None]
            last_rn = (t_steps - 1) // RN * RN  # last renorm step < t_steps
            CW = HCN * BG                      # rb width per chain (16)

            def rgroup(g, u):
                # column sums of chain g's phat_u into PSUM r-strip slot u%RN
                nonlocal rps
                if u % RN == 0 and g == 0:
                    rps = rp.tile([1, RN * BL], F32, tag="rstrip")
                lo = (u % RN) * BL + g * BG
                for jc in range(HCN):
                    nc.tensor.matmul(rps[:, lo:lo + BG],
                                     lhsT=ones128[:, :],
                                     rhs=phat[g][:, jc * BG:(jc + 1) * BG],
                                     start=(jc == 0), stop=(jc == HCN - 1))

            for t in range(1, t_steps):
                blk = t // TBLK
                j = t % TBLK
                m = t % RN
                tr = t - m + RN          # next renorm step after t
                prep = (m >= 2 and tr <= last_rn)

                # PE: q_g = P^T phat_g (16 matmuls each), then column sums
                qs = []
                for g in range(NG):
                    q = qp.tile([P_, HCN * BG], F32, tag=f"q{g}")
                    for kc in range(HCN):
                        for jc in range(HCN):
                            nc.tensor.matmul(
                                q[:, kc * BG:(kc + 1) * BG],
                                lhsT=pm_t[:, (jc * HCN + kc) * P_:
                                          (jc * HCN + kc + 1) * P_],
                                rhs=phat[g][:, jc * BG:(jc + 1) * BG],
                                start=(jc == 0), stop=(jc == HCN - 1))
                    qs.append(q)
                    rgroup(g, t - 1)
                if (t - 1) % RN == RN - 1:
                    grp = (t - 1) // RN
                    nc.scalar.copy(
                        rstrip_t[:, grp * RN * BL:(grp + 1) * RN * BL],
                        rps[:, :])
                # PE (off-chain): renorm scale source = column sums of phat
                if prep and m == 2:
                    combo = cbp.tile([P_, NG * CW + NG * BG], F32, tag="combo")
                    for g in range(NG):
                        lo = NG * CW + g * BG
                        for jc in range(HCN):
                            nc.tensor.matmul(
                                combo[0:1, lo:lo + BG], lhsT=ones128[:, :],
                                rhs=phat[g][:, jc * BG:(jc + 1) * BG],
                                start=(jc == 0), stop=(jc == HCN - 1))
                # PE (off-chain): broadcast rinv over partitions
                if prep and m == 5:
                    for g in range(NG):
                        nc.tensor.matmul(combo[:, g * CW:(g + 1) * CW],
                                         lhsT=onesrow_f[:, :],
                                         rhs=tiled[:, g * CW:(g + 1) * CW],
                                         start=True, stop=True)
                # Pool: prefetch gathers two blocks ahead
                if j == 14 and blk + 2 < nblk:
                    emit_gather(blk + 2)
                # PE/Act (off-chain): transpose+exp bursts for next block
                if blk + 1 < nblk and 7 <= j <= 10:
                    emit_chunk(blk + 1, j - 7)

                # DVE: the chain multiplies
                for g in range(NG):
                    pnew = pp.tile([P_, HCN * BG], BF16, tag=f"ph{g}")
                    pv = pnew.rearrange("p (c b) -> p c b", c=HCN)
                    qv = qs[g].rearrange("p (c b) -> p c b", c=HCN)
                    if m == 0 and ebr_cur[g] is not None:
                        ev = ebr_cur[g].rearrange("p (c b) -> p c b", c=HCN)
                        ebr_cur[g] = None
                    else:
                        ev = eb_slice(t, g)
                    nc.vector.tensor_tensor(pv[:, :, :], qv[:, :, :],
                                            ev[:, :, :], MULT)
                    phat[g] = pnew

                # DVE/Act (off-chain): renorm preparation pipeline
                if prep and m == 3:
                    rv8 = sp.tile([1, BL], F32, tag="rv8")
                    nc.vector.reciprocal(rv8[:, :],
                                         combo[0:1, NG * CW:NG * CW + BL])
                    nc.scalar.copy(rinv_t[:, ridx * BL:(ridx + 1) * BL],
                                   rv8[:, :])
                    ridx += 1
                    tiled = sp.tile([1, NG * CW], F32, tag="tiled")
                    for g in range(NG):
                        o = g * CW
                        nc.scalar.copy(tiled[:, o:o + BG],
                                       rv8[:, g * BG:(g + 1) * BG])
                        nc.scalar.copy(tiled[:, o + BG:o + 2 * BG],
                                       tiled[:, o:o + BG])
                        nc.scalar.copy(tiled[:, o + 2 * BG:o + 4 * BG],
                                       tiled[:, o:o + 2 * BG])
                if prep and m >= 6:
                    g = m - 6
                    ebr = er.tile([P_, HCN * BG], BF16, tag=f"ebr{g}")
                    rbv = combo[:, g * CW:(g + 1) * CW].rearrange(
                        "p (c b) -> p c b", c=HCN)
                    nc.vector.tensor_tensor(
                        ebr.rearrange("p (c b) -> p c b", c=HCN)[:, :, :],
                        eb_slice(tr, g)[:, :, :], rbv[:, :, :], MULT)
                    ebr_cur[g] = ebr

            for g in range(NG):
                rgroup(g, t_steps - 1)
            grp = (t_steps - 1) // RN
            nc.scalar.copy(rstrip_t[:, grp * RN * BL:(grp + 1) * RN * BL],
                           rps[:, :])
            nc.sync.dma_start(rstrip_d[:, :], rstrip_t[:, :])
            nc.sync.dma_start(rinv_d[:, :], rinv_t[:, :])

    nc.compile()
    return nc


def _get_compiled(t_steps=T):
    if t_steps not in _compiled:
        _compiled[t_steps] = build(t_steps)
    return _compiled[t_steps]


def _host_prep(obs, emis, tran, priors, t_steps):
    """Returns (shared_inputs, per_core_idx, kappa)."""
    nblk = t_steps // TBLK
    # transition softmax -> bf16 chunk layout [j, (jc*HCN+kc)*128 + k]
    m = tran.max(axis=1, keepdims=True)
    e = np.exp(tran - m, dtype=np.float32)
    P = (e / e.sum(axis=1, keepdims=True)).astype(ml_dtypes.bfloat16)
    pm = np.ascontiguousarray(
        P.reshape(HCN, P_, HCN, P_).transpose(1, 0, 2, 3).reshape(P_, -1))

    # transposed bf16 emission table, rows indexed by s*V+v
    tabT = np.ascontiguousarray(
        emis.transpose(0, 2, 1)).astype(ml_dtypes.bfloat16).reshape(S * V, H)

    # L[h] and kappa
    mx = emis.max(axis=2)                                   # (S,H)
    lse = mx + np.log(np.exp(emis - mx[:, :, None],
                             dtype=np.float32).sum(axis=2))
    L = 0.25 * lse.sum(axis=0)                              # (H,)
    kap_h = 0.25 * mx.sum(axis=0) - L
    kappa = float(kap_h.max())
    bias = np.ascontiguousarray(
        (-(L + kappa)).astype(np.float32).reshape(HCN, P_).T)   # (128,4)
    bias2 = np.ascontiguousarray(
        (-(L + kappa) + priors).astype(np.float32).reshape(HCN, P_).T)

    # per-core gather row indices: idx[p=(tt*BL+bb), s*nblk+blk]
    per_core_idx = []
    svec = (np.arange(S, dtype=np.int64) * V)
    for c in range(NC):
        o = obs[c * BL:(c + 1) * BL, :t_steps, :]           # (BL,t,S)
        o = o + svec[None, None, :]
        o = o.transpose(1, 0, 2)                            # (t, BL, S)
        o = o.reshape(nblk, TBLK, BL, S)
        o = o.transpose(1, 2, 3, 0).reshape(TBLK * BL, S * nblk)
        per_core_idx.append(np.ascontiguousarray(o.astype(np.int32)))

    shared = {"tabt": tabT, "pm": pm, "bias": bias, "bias2": bias2}
    return shared, per_core_idx, kappa


def _host_post(results, lengths, kappa, t_steps):
    nrn = max(1, _n_renorms(t_steps))
    ans = np.zeros((B, 1), np.float32)
    tt = np.arange(t_steps, dtype=np.float64)
    for c in range(NC):
        r = results[c]["rstrip"].reshape(t_steps, BL).astype(np.float64)
        rinv = results[c]["rinvstrip"].reshape(nrn, BL).astype(np.float64)
        rho_log = np.zeros((t_steps, BL), np.float64)
        k = 0
        for t in range(1, t_steps):
            if t % RN == 0:
                rho_log[t] = np.log(rinv[k])
                k += 1
        logsums = np.log(r) + (tt[:, None] + 1.0) * kappa \
            - np.cumsum(rho_log, axis=0)
        lens = np.clip(lengths[c * BL:(c + 1) * BL], 1, t_steps)
        ans[c * BL:(c + 1) * BL, 0] = logsums[
            lens - 1, np.arange(BL)].astype(np.float32)
    return ans


def run(inputs, t_steps=T, trace=False):
    obs = np.asarray(inputs["obs"])
    lengths = np.asarray(inputs["lengths"])
    emis = np.asarray(inputs["unnormalized_emis"], np.float32)
    tran = np.asarray(inputs["unnormalized_tran"], np.float32)
    priors = np.asarray(inputs["log_state_priors"], np.float32)

    nc = _get_compiled(t_steps)
    shared, per_core_idx, kappa = _host_prep(obs, emis, tran, priors, t_steps)
    in_maps = [dict(shared, idx=per_core_idx[c]) for c in range(NC)]
    res = bass_utils.run_bass_kernel_spmd(nc, in_maps,
                                          core_ids=list(range(NC)),
                                          trace=trace)
    ans = _host_post(res.results, lengths, kappa, t_steps)
    return ans, res


def kernel(obs, lengths, unnormalized_emis, unnormalized_tran,
           log_state_priors):
    ans, _ = run(dict(obs=obs, lengths=lengths,
                      unnormalized_emis=unnormalized_emis,
                      unnormalized_tran=unnormalized_tran,
                      log_state_priors=log_state_priors))
    return ans


# revision 9
# speedup vs baseline: 1.5998x; 1.0153x over previous
"""Trainium2 Bass kernel for the HMM forward-algorithm problem.

Strategy
--------
The reference does, per time step, a log-domain matrix-vector product
  alpha_t[b,k] = em[b,t,k] + logsumexp_j(alpha_{t-1}[b,j] + tran[j,k])
followed by logsumexp_k.  We run the whole recurrence in *probability*
domain on the TensorEngine:

  phat_t = E_t  *  (phat_{t-1} @ P)          (elementwise * matmul)

where P = softmax(tran) rows (constant) and E_t = exp(em_t - kappa) with a
global shift kappa that keeps E <= ~1.  phat decays by ~e^-3 per step, so we
renormalise every RN steps by an earlier column sum (dumping the exact f32
scale used so the host can undo it).

The recurrence is a T-link serial chain PE -> (PSUM latency) -> DVE multiply
-> (latency) -> PE whose per-link latency is fixed-cost dominated, so the 8
batch rows per core are split into TWO independent 4-row chains that
interleave: each chain's link is cheaper and the engines stay busy with the
other chain during latency gaps.  Everything else is kept OFF the chains:

- renorm: the reciprocal/broadcast/E-scale are prepared 5+ steps ahead and
  folded into a pre-scaled E-strip slice, so renorm steps cost nothing;
- per-step column sums (the per-t logsumexp output) accumulate into a PSUM
  strip of RN slots, copied out by the Act engine once per RN steps;
- emission gathers: indirect DMA fetches bf16 rows two blocks ahead; the 4
  sources are summed via matmul-by-identity transposes accumulating in PSUM
  (PE idle windows), then Act applies exp(0.25*x - L - kappa) into the
  E-strip.

Emissions: em[b,t,h] = 0.25 * sum_s x[s,h,obs[b,t,s]] - L[h], where
x is the raw emission table and L[h] = 0.25*sum_s logsumexp_v x[s,h,:].
The host pre-transposes x to a (S*V, H) bf16 row table; the device gathers
rows with indirect DMA (128 rows = 16 timesteps x 8 batch per source).

Sharding: data-parallel over batch (8 of 64 rows per core).  Tables are
replicated.  No collectives.  Final log / cumsum / length-indexing is tiny
(T x B) and done on the host in float64.
"""
import sys

sys.path.insert(0, "/opt/trn_rl_repo")

import numpy as np
import ml_dtypes

import concourse.bass as bass
import concourse.bacc as bacc
import concourse.tile as tile
import concourse.mybir as mybir
import concourse.bass_utils as bass_utils
from concourse.masks import make_identity

B, T, S, H, V = 64, 512, 4, 512, 10000
NC = 8            # cores
BL = B // NC      # batch rows per core
NG = 2            # independent chains per core
BG = BL // NG     # batch rows per chain
P_ = 128          # partitions
HCN = H // P_     # h chunks
TBLK = 16         # timesteps per gather block
RN = 8            # renorm interval
F32 = mybir.dt.float32
BF16 = mybir.dt.bfloat16
I32 = mybir.dt.int32
EXP = mybir.ActivationFunctionType.Exp
MULT = mybir.AluOpType.mult

_compiled = {}


def _n_renorms(t_steps):
    return len([t for t in range(1, t_steps) if t % RN == 0])


def build(t_steps=T):
    """Build + bacc-compile the per-core Bass program (identical on all cores)."""
    nblk = t_steps // TBLK
    nc = bacc.Bacc("TRN2", target_bir_lowering=False, debug=False,
                   enable_asserts=False, num_devices=NC)

    tabt = nc.dram_tensor("tabt", [S * V, H], BF16, kind="ExternalInput").ap()
    pm_d = nc.dram_tensor("pm", [P_, HCN * HCN * P_], BF16, kind="ExternalInput").ap()
    idx_d = nc.dram_tensor("idx", [P_, S * nblk], I32, kind="ExternalInput").ap()
    bias_d = nc.dram_tensor("bias", [P_, HCN], F32, kind="ExternalInput").ap()
    expp_d = nc.dram_tensor("expp", [P_, HCN], F32, kind="ExternalInput").ap()
    rstrip_d = nc.dram_tensor("rstrip", [1, t_steps * BL], F32,
                              kind="ExternalOutput").ap()
    nrn = max(1, _n_renorms(t_steps))
    rinv_d = nc.dram_tensor("rinvstrip", [1, nrn * BL], F32,
                            kind="ExternalOutput").ap()

    with tile.TileContext(nc) as tc:
        with (tc.tile_pool(name="const", bufs=1) as cp,
              tc.tile_pool(name="estrip", bufs=nblk) as ep,
              tc.tile_pool(name="gath", bufs=9) as gp,
              tc.tile_pool(name="phat", bufs=3) as pp,
              tc.tile_pool(name="small", bufs=2) as sp,
              tc.tile_pool(name="ebr", bufs=2) as er,
              tc.tile_pool(name="qpsum", bufs=2, space="PSUM") as qp,
              tc.tile_pool(name="rstripps", bufs=2, space="PSUM") as rp,
              tc.tile_pool(name="combops", bufs=1, space="PSUM") as cbp,
              tc.tile_pool(name="tpsum", bufs=1, space="PSUM") as tp_):

            # ---- constants ----
            idx_t = cp.tile([P_, S * nblk], I32, name="idxt")
            nc.sync.dma_start(idx_t[:, :], idx_d[:, :])
            pm_t = cp.tile([P_, HCN * HCN * P_], BF16, name="pmt")
            nc.sync.dma_start(pm_t[:, :], pm_d[:, :])
            bias_t = cp.tile([P_, HCN], F32, name="biast")
            nc.sync.dma_start(bias_t[:, :], bias_d[:, :])
            expp_t = cp.tile([P_, HCN], F32, name="exppt")
            nc.sync.dma_start(expp_t[:, :], expp_d[:, :])
            ones128 = cp.tile([P_, 1], BF16, name="ones128")
            nc.gpsimd.memset(ones128[:, :], 1.0)
            onesrow_f = cp.tile([1, P_], F32, name="onesrowf")
            nc.gpsimd.memset(onesrow_f[:, :], 1.0)
            identb = cp.tile([P_, P_], BF16, name="identb")
            make_identity(nc, identb[:, :])
            rstrip_t = cp.tile([1, t_steps * BL], F32, name="rstript")
            rinv_t = cp.tile([1, nrn * BL], F32, name="rinvt")

            eb_list = [None] * nblk
            g_list = [None] * nblk

            def emit_gather(blk):
                gs = []
                for s in range(S):
                    g = gp.tile([P_, H], BF16, tag="g", name=f"g{blk}_{s}")
                    col = s * nblk + blk
                    nc.gpsimd.indirect_dma_start(
                        out=g[:, :], out_offset=None, in_=tabt[:, :],
                        in_offset=bass.IndirectOffsetOnAxis(
                            ap=idx_t[:, col:col + 1], axis=0))
                    gs.append(g)
                g_list[blk] = gs
                eb_list[blk] = ep.tile([P_, TBLK * HCN * BL], BF16, tag="eb",
                                       name=f"eb{blk}")

            def emit_chunk(blk, c):
                # transpose the 4 source gathers for h-chunk c, summing in
                # PSUM, then exp into the E-strip on the Act engine
                gs = g_list[blk]
                tpp = tp_.tile([P_, P_], F32, tag="tp")
                for s in range(S):
                    nc.tensor.matmul(tpp[:, :],
                                     lhsT=gs[s][:, c * P_:(c + 1) * P_],
                                     rhs=identb[:, :],
                                     start=(s == 0), stop=(s == S - 1))
                eb4 = eb_list[blk].rearrange("p (t c b) -> p t c b",
                                             t=TBLK, c=HCN)
                nc.scalar.activation(
                    eb4[:, :, c, :],
                    tpp.rearrange("p (t b) -> p t b", t=TBLK),
                    EXP, bias=bias_t[:, c:c + 1], scale=0.25)
                return tpp

            def eb_slice(t, g):
                # [128, (HCN, BG)] E-strip view for chain g at step t
                eb4 = eb_list[t // TBLK].rearrange("p (t c b) -> p t c b",
                                                   t=TBLK, c=HCN)
                return eb4[:, t % TBLK, :, g * BG:(g + 1) * BG]

            # ---- blocks 0/1: gathers; block 0 transposes + phat_0 ----
            emit_gather(0)
            emit_gather(1)
            phat = [pp.tile([P_, HCN * BG], BF16, tag=f"ph{g}",
                            name=f"phat0_{g}") for g in range(NG)]
            tpp0 = [tp_.tile([P_, P_], F32, tag="tp", name="tpp0_0"),
                    qp.tile([P_, P_], F32, tag="q0", name="tpp0_1"),
                    qp.tile([P_, P_], F32, tag="q1", name="tpp0_2"),
                    cbp.tile([P_, P_], F32, tag="combo", name="tpp0_3")]
            for s_ in range(S):
                for c in range(HCN):
                    nc.tensor.matmul(tpp0[c][:, :],
                                     lhsT=g_list[0][s_][:, c * P_:(c + 1) * P_],
                                     rhs=identb[:, :],
                                     start=(s_ == 0), stop=(s_ == S - 1))
            eb4_0 = eb_list[0].rearrange("p (t c b) -> p t c b", t=TBLK, c=HCN)
            for c in range(HCN):
                nc.scalar.activation(
                    eb4_0[:, :, c, :],
                    tpp0[c].rearrange("p (t b) -> p t b", t=TBLK),
                    EXP, bias=bias_t[:, c:c + 1], scale=0.25)
                for g in range(NG):
                    nc.vector.tensor_scalar_mul(
                        phat[g][:, c * BG:(c + 1) * BG],
                        eb4_0[:, 0, c, g * BG:(g + 1) * BG],
                        expp_t[:, c:c + 1])

            # ---- interleaved gather + two-chain scan ----
            # combo PSUM tile columns: rb_g at [g*16:(g+1)*16), r2_g at
            # [32+g*4 : 32+(g+1)*4) on partition 0
            ridx = 0
            rps = None
            combo = None
            tiled = None
            rv8 = None
            ebr_cur = [None, None]
            last_rn = (t_steps - 1) // RN * RN  # last renorm step < t_steps
            CW = HCN * BG                      # rb width per chain (16)

            def rgroup(g, u):
                # column sums of chain g's phat_u into PSUM r-strip slot u%RN
                nonlocal rps
                if u % RN == 0 and g == 0:
                    rps = rp.tile([1, RN * BL], F32, tag="rstrip")
                lo = (u % RN) * BL + g * BG
                for jc in range(HCN):
                    nc.tensor.matmul(rps[:, lo:lo + BG],
                                     lhsT=ones128[:, :],
                                     rhs=phat[g][:, jc * BG:(jc + 1) * BG],
                                     start=(jc == 0), stop=(jc == HCN - 1))

            for t in range(1, t_steps):
                blk = t // TBLK
                j = t % TBLK
                m = t % RN
                tr = t - m + RN          # next renorm step after t
                prep = (m >= 2 and tr <= last_rn)

                # PE: q_g = P^T phat_g (16 matmuls each), then column sums
                qs = []
                for g in range(NG):
                    q = qp.tile([P_, HCN * BG], F32, tag=f"q{g}")
                    for kc in range(HCN):
                        for jc in range(HCN):
                            nc.tensor.matmul(
                                q[:, kc * BG:(kc + 1) * BG],
                                lhsT=pm_t[:, (jc * HCN + kc) * P_:
                                          (jc * HCN + kc + 1) * P_],
                                rhs=phat[g][:, jc * BG:(jc + 1) * BG],
                                start=(jc == 0), stop=(jc == HCN - 1))
                    qs.append(q)
                    rgroup(g, t - 1)
                if (t - 1) % RN == RN - 1:
                    grp = (t - 1) // RN
                    nc.scalar.copy(
                        rstrip_t[:, grp * RN * BL:(grp + 1) * RN * BL],
                        rps[:, :])
                # PE (off-chain): renorm scale source = column sums of phat
                if prep and m == 2:
                    combo = cbp.tile([P_, NG * CW + NG * BG], F32, tag="combo")
                    for g in range(NG):
                        lo = NG * CW + g * BG
                        for jc in range(HCN):
                            nc.tensor.matmul(
                                combo[0:1, lo:lo + BG], lhsT=ones128[:, :],
                                rhs=phat[g][:, jc * BG:(jc + 1) * BG],
                                start=(jc == 0), stop=(jc == HCN - 1))
                # PE (off-chain): broadcast rinv over partitions
                if prep and m == 5:
                    for g in range(NG):
                        nc.tensor.matmul(combo[:, g * CW:(g + 1) * CW],
                                         lhsT=onesrow_f[:, :],
                                         rhs=tiled[:, g * CW:(g + 1) * CW],
                                         start=True, stop=True)
                # Pool: prefetch gathers two blocks ahead
                if j == 14 and blk + 2 < nblk:
                    emit_gather(blk + 2)
                # PE/Act (off-chain): transpose+exp bursts for next block
                if blk + 1 < nblk and 7 <= j <= 10:
                    emit_chunk(blk + 1, j - 7)

                # DVE: the chain multiplies
                for g in range(NG):
                    pnew = pp.tile([P_, HCN * BG], BF16, tag=f"ph{g}")
                    pv = pnew.rearrange("p (c b) -> p c b", c=HCN)
                    qv = qs[g].rearrange("p (c b) -> p c b", c=HCN)
                    if m == 0 and ebr_cur[g] is not None:
                        ev = ebr_cur[g].rearrange(
                            "p (c b) -> p c b", c=HCN)[:, :, g * BG:(g + 1) * BG]
                        ebr_cur[g] = None
                    else:
                        ev = eb_slice(t, g)
                    nc.vector.tensor_tensor(pv[:, :, :], qv[:, :, :],
                                            ev[:, :, :], MULT)
                    phat[g] = pnew

                # DVE/Act (off-chain): renorm preparation pipeline
                if prep and m == 3:
                    rv8 = sp.tile([1, BL], F32, tag="rv8")
                    nc.vector.reciprocal(rv8[:, :],
                                         combo[0:1, NG * CW:NG * CW + BL])
                    nc.scalar.copy(rinv_t[:, ridx * BL:(ridx + 1) * BL],
                                   rv8[:, :])
                    ridx += 1
                    tiled = sp.tile([1, NG * CW], F32, tag="tiled")
                    for g in range(NG):
                        o = g * CW
                        nc.scalar.copy(tiled[:, o:o + BG],
                                       rv8[:, g * BG:(g + 1) * BG])
                        nc.scalar.copy(tiled[:, o + BG:o + 2 * BG],
                                       tiled[:, o:o + BG])
                        nc.scalar.copy(tiled[:, o + 2 * BG:o + 4 * BG],
                                       tiled[:, o:o + 2 * BG])
                if prep and m == 6:
                    ebr = er.tile([P_, HCN * BL], BF16, tag="ebr")
                    cv = combo[:, 0:NG * CW].rearrange(
                        "p (g c b) -> p c g b", g=NG, c=HCN)
                    eb4 = eb_list[tr // TBLK].rearrange(
                        "p (t c b) -> p t c b", t=TBLK, c=HCN)
                    e4 = eb4[:, tr % TBLK, :, :].rearrange(
                        "p c (g b) -> p c g b", g=NG)
                    o4 = ebr.rearrange("p (c g b) -> p c g b", c=HCN, g=NG)
                    nc.vector.tensor_tensor(o4[:, :, :, :], e4[:, :, :, :],
                                            cv[:, :, :, :], MULT)
                    ebr_cur = [ebr, ebr]

            for g in range(NG):
                rgroup(g, t_steps - 1)
            grp = (t_steps - 1) // RN
            nc.scalar.copy(rstrip_t[:, grp * RN * BL:(grp + 1) * RN * BL],
                           rps[:, :])
            nc.sync.dma_start(rstrip_d[:, :], rstrip_t[:, :])
            nc.sync.dma_start(rinv_d[:, :], rinv_t[:, :])

    nc.compile()
    return nc


def _get_compiled(t_steps=T):
    if t_steps not in _compiled:
        _compiled[t_steps] = build(t_steps)
    return _compiled[t_steps]


def _host_prep(obs, emis, tran, priors, t_steps):
    """Returns (shared_inputs, per_core_idx, kappa)."""
    nblk = t_steps // TBLK
    # transition softmax -> bf16 chunk layout [j, (jc*HCN+kc)*128 + k]
    m = tran.max(axis=1, keepdims=True)
    e = np.exp(tran - m, dtype=np.float32)
    P = (e / e.sum(axis=1, keepdims=True)).astype(ml_dtypes.bfloat16)
    pm = np.ascontiguousarray(
        P.reshape(HCN, P_, HCN, P_).transpose(1, 0, 2, 3).reshape(P_, -1))

    # transposed bf16 emission table, rows indexed by s*V+v
    tabT = np.ascontiguousarray(
        emis.transpose(0, 2, 1)).astype(ml_dtypes.bfloat16).reshape(S * V, H)

    # L[h] and kappa
    mx = emis.max(axis=2)                                   # (S,H)
    lse = mx + np.log(np.exp(emis - mx[:, :, None],
                             dtype=np.float32).sum(axis=2))
    L = 0.25 * lse.sum(axis=0)                              # (H,)
    kap_h = 0.25 * mx.sum(axis=0) - L
    kappa = float(kap_h.max())
    bias = np.ascontiguousarray(
        (-(L + kappa)).astype(np.float32).reshape(HCN, P_).T)   # (128,4)
    expp = np.ascontiguousarray(
        np.exp(priors, dtype=np.float32).reshape(HCN, P_).T)

    # per-core gather row indices: idx[p=(tt*BL+bb), s*nblk+blk]
    per_core_idx = []
    svec = (np.arange(S, dtype=np.int64) * V)
    for c in range(NC):
        o = obs[c * BL:(c + 1) * BL, :t_steps, :]           # (BL,t,S)
        o = o + svec[None, None, :]
        o = o.transpose(1, 0, 2)                            # (t, BL, S)
        o = o.reshape(nblk, TBLK, BL, S)
        o = o.transpose(1, 2, 3, 0).reshape(TBLK * BL, S * nblk)
        per_core_idx.append(np.ascontiguousarray(o.astype(np.int32)))

    shared = {"tabt": tabT, "pm": pm, "bias": bias, "expp": expp}
    return shared, per_core_idx, kappa


def _host_post(results, lengths, kappa, t_steps):
    nrn = max(1, _n_renorms(t_steps))
    ans = np.zeros((B, 1), np.float32)
    tt = np.arange(t_steps, dtype=np.float64)
    for c in range(NC):
        r = results[c]["rstrip"].reshape(t_steps, BL).astype(np.float64)
        rinv = results[c]["rinvstrip"].reshape(nrn, BL).astype(np.float64)
        rho_log = np.zeros((t_steps, BL), np.float64)
        k = 0
        for t in range(1, t_steps):
            if t % RN == 0:
                rho_log[t] = np.log(rinv[k])
                k += 1
        logsums = np.log(r) + (tt[:, None] + 1.0) * kappa \
            - np.cumsum(rho_log, axis=0)
        lens = np.clip(lengths[c * BL:(c + 1) * BL], 1, t_steps)
        ans[c * BL:(c + 1) * BL, 0] = logsums[
            lens - 1, np.arange(BL)].astype(np.float32)
    return ans


def run(inputs, t_steps=T, trace=False):
    obs = np.asarray(inputs["obs"])
    lengths = np.asarray(inputs["lengths"])
    emis = np.asarray(inputs["unnormalized_emis"], np.float32)
    tran = np.asarray(inputs["unnormalized_tran"], np.float32)
    priors = np.asarray(inputs["log_state_priors"], np.float32)

    nc = _get_compiled(t_steps)
    shared, per_core_idx, kappa = _host_prep(obs, emis, tran, priors, t_steps)
    in_maps = [dict(shared, idx=per_core_idx[c]) for c in range(NC)]
    res = bass_utils.run_bass_kernel_spmd(nc, in_maps,
                                          core_ids=list(range(NC)),
                                          trace=trace)
    ans = _host_post(res.results, lengths, kappa, t_steps)
    return ans, res


def kernel(obs, lengths, unnormalized_emis, unnormalized_tran,
           log_state_priors):
    ans, _ = run(dict(obs=obs, lengths=lengths,
                      unnormalized_emis=unnormalized_emis,
                      unnormalized_tran=unnormalized_tran,
                      log_state_priors=log_state_priors))
    return ans
